# revision 17
# baseline (speedup 1.0000x reference)
import sys

import numpy as np

sys.path.insert(0, "/opt/trn_rl_repo")

import ml_dtypes  # noqa: E402

import concourse.bacc as bacc  # noqa: E402
import concourse.bass as bass  # noqa: E402
import concourse.tile as tile  # noqa: E402
from concourse import masks, mybir  # noqa: E402
from concourse.bass_utils import run_bass_kernel_spmd  # noqa: E402

C, H, W = 512, 64, 64
HW = H * W          # 4096
C8 = 64             # pos-att channels
NCORE = 8
IB = HW // NCORE    # 512 spatial rows of A per core
CH = C // NCORE     # 64 channels per core
H2 = W2 = 32
PIX = H2 * W2       # 1024
SCALE = 32.0        # fp8 range scaling for the P matmul
F32 = mybir.dt.float32
BF16 = mybir.dt.bfloat16
FP8 = mybir.dt.float8e4
AX = mybir.AxisListType.X
OP = mybir.AluOpType
AF = mybir.ActivationFunctionType
PM = mybir.MatmulPerfMode

_BF = ml_dtypes.bfloat16


def _bcast(ap, pos, n):
    """Insert a stride-0 (broadcast) free dim of size n at free position pos."""
    a = [list(d) for d in ap.ap]
    a.insert(1 + pos, [0, n])
    return bass.AP(tensor=ap.tensor, offset=ap.offset, ap=a)


def _unit(ap):
    """Append a trailing unit free dim (for reduce outputs)."""
    a = [list(d) for d in ap.ap] + [[0, 1]]
    return bass.AP(tensor=ap.tensor, offset=ap.offset, ap=a)


def _build_real():
    nc = bacc.Bacc()

    xhw = nc.declare_dram_parameter("xhw", [C, HW], BF16, isOutput=False)
    xP = nc.declare_dram_parameter("xP", [C, IB], BF16, isOutput=False)
    x5T2 = nc.declare_dram_parameter("x5T2", [2, 128, 2, C], BF16,
                                     isOutput=False)
    xblk = nc.declare_dram_parameter("xblk", [CH, HW], F32, isOutput=False)
    wposT = nc.declare_dram_parameter("wposT", [C, C8], BF16, isOutput=False)
    bpos = nc.declare_dram_parameter("bpos", [C8, 1], F32, isOutput=False)
    w3a = nc.declare_dram_parameter("w3a", [4, 128, 9, CH], BF16,
                                    isOutput=False)
    w3b = nc.declare_dram_parameter("w3b", [CH, 9, C], BF16, isOutput=False)
    b3 = nc.declare_dram_parameter("b3", [CH, 1], F32, isOutput=False)
    out_ext = nc.declare_dram_parameter("out", [CH, HW], BF16, isOutput=True)

    warm_in = nc.dram_tensor("warm_in", [NCORE, 64], BF16)
    warm_out = nc.dram_tensor("warm_out", [1, 64], BF16)
    p_drams = []
    p_rss = []
    for i, sz in enumerate([1024, 1024, 2048]):
        pd = nc.dram_tensor(f"p_bounce{i}", [C, sz], BF16)
        p_drams.append(pd)
        pr = nc.dram_tensor(f"p_rs{i}", [CH, sz], BF16)
        p_rss.append(pr)
    x12ds = []
    x12rss = []
    for i in range(2):
        xd = nc.dram_tensor(f"x12_bounce{i}", [C, 512], BF16)
        x12ds.append(xd)
        xr = nc.dram_tensor(f"x12_rs{i}", [CH, 512], BF16)
        x12rss.append(xr)
    bT_dram = nc.dram_tensor("bT_dram", [CH, PIX], BF16)

    groups = [list(range(NCORE))]
    taps = [(1, 1)] + [(kh, kw) for kh in range(3) for kw in range(3)
                       if (kh, kw) != (1, 1)]

    with tile.TileContext(nc) as tc, \
         tc.tile_pool(name="big", bufs=1) as big, \
         tc.tile_pool(name="sm", bufs=1) as sm, \
         tc.tile_pool(name="stg", bufs=4) as stg, \
         tc.tile_pool(name="stat", bufs=2) as stat:

        # Warm-up collective with no data deps: posts immediately, absorbs
        # the comm-init barrier + inter-core launch skew off the critical
        # path (the first real collective otherwise pays it).
        nc.gpsimd.collective_compute(
            "ReduceScatter", OP.add, replica_groups=groups,
            ins=[warm_in[:, :]], outs=[warm_out[:, :]])

        # ---------- loads (in consumption order) ----------
        wp = []
        xsb = []
        for k in range(4):
            t = sm.tile([128, C8], BF16, tag=f"wp{k}")
            nc.sync.dma_start(out=t[:, :], in_=wposT[k * 128:(k + 1) * 128, :])
            wp.append(t)
            t = big.tile([128, HW], BF16, tag=f"xsb{k}")
            nc.sync.dma_start(out=t[:, :2048],
                              in_=xhw[k * 128:(k + 1) * 128, :2048])
            nc.sync.dma_start(out=t[:, 2048:],
                              in_=xhw[k * 128:(k + 1) * 128, 2048:])
            xsb.append(t)
        bpos_sb = sm.tile([C8, 1], F32, tag="bpos")
        nc.sync.dma_start(out=bpos_sb[:, :], in_=bpos[:, :])
        xp = []
        for k in range(4):
            t = sm.tile([128, IB], BF16, tag=f"xp{k}")
            nc.sync.dma_start(out=t[:, :], in_=xP[k * 128:(k + 1) * 128, :])
            xp.append(t)
        x5t2 = []
        for p in range(2):
            t = sm.tile([128, 2, C], BF16, tag=f"x5t2_{p}")
            nc.sync.dma_start(out=t[:, :, :], in_=x5T2[p, :, :, :])
            x5t2.append(t)
        w3sb = []
        for k in range(4):
            t = sm.tile([128, 9, CH], BF16, tag=f"w3a{k}")
            nc.sync.dma_start(out=t[:, :, :], in_=w3a[k, :, :, :])
            w3sb.append(t)
        b3_sb = sm.tile([CH, 1], F32, tag="b3")
        nc.sync.dma_start(out=b3_sb[:, :], in_=b3[:, :])
        xblk_sb = big.tile([CH, HW], F32, tag="xblk")
        nc.sync.dma_start(out=xblk_sb[:, :], in_=xblk[:, :])
        w3b_sb = sm.tile([CH, 9, C], BF16, tag="w3b")
        nc.sync.dma_start(out=w3b_sb[:, :, :], in_=w3b[:, :, :])

        A8 = []
        x5t8 = []
        for p in range(2):
            a8t = big.tile([128, 2, HW], FP8, tag=f"A8_{p}")
            A8.append(a8t)
            x58t = sm.tile([128, 2, C], FP8, tag=f"x5t8_{p}")
            x5t8.append(x58t)

        with tc.tile_pool(name="ps8", bufs=8, space="PSUM") as ps:
            # ---------- x3f = w_pos @ x_hw + b  (C8, HW), bf16 ----------
            x3f = big.tile([C8, HW], BF16, tag="x3f")
            for njj in range(8):
                pt = ps.tile([128, 512], F32, tag="ps")
                for k in range(4):
                    nc.tensor.matmul(
                        pt[:C8, :], wp[k][:, :],
                        xsb[k][:, njj * 512:(njj + 1) * 512],
                        start=(k == 0), stop=(k == 3))
                nc.vector.tensor_scalar_add(
                    out=x3f[:, njj * 512:(njj + 1) * 512], in0=pt[:C8, :],
                    scalar1=bpos_sb[:, :])

            # ---------- x3b = w_pos @ xP + b  (C8, IB) ----------
            x3b = sm.tile([C8, IB], BF16, tag="x3b")
            pt = ps.tile([128, 512], F32, tag="ps")
            for k in range(4):
                nc.tensor.matmul(
                    pt[:C8, :], wp[k][:, :], xp[k][:, :],
                    start=(k == 0), stop=(k == 3))
            nc.vector.tensor_scalar_add(
                out=x3b[:, :], in0=pt[:C8, :], scalar1=bpos_sb[:, :])

            # ---------- A rows + softmax -> A8 (fp8), x5 scale ----------
            for mi in range(4):
                pr, sub = divmod(mi, 2)
                mx8 = stat.tile([128, 8], F32, tag="mx8")
                pts = []
                for njj in range(8):
                    pt = ps.tile([128, 512], F32, tag="ps")
                    nc.tensor.matmul(
                        pt[:, :], x3b[:, mi * 128:(mi + 1) * 128],
                        x3f[:, njj * 512:(njj + 1) * 512],
                        start=True, stop=True)
                    nc.vector.reduce_max(
                        out=mx8[:, njj:njj + 1], in_=pt[:, :], axis=AX)
                    pts.append(pt)
                mxn = stat.tile([128, 1], F32, tag="mxn")
                nc.vector.reduce_max(out=mxn[:, :], in_=mx8[:, :], axis=AX)
                nc.vector.tensor_scalar_mul(out=mxn[:, :], in0=mxn[:, :],
                                            scalar1=-1.0)
                s8 = stat.tile([128, 8], F32, tag="s8")
                for njj in range(8):
                    nc.scalar.activation(
                        out=A8[pr][:, sub, njj * 512:(njj + 1) * 512],
                        in_=pts[njj][:, :],
                        func=AF.Exp, bias=mxn[:, :], scale=1.0,
                        accum_out=s8[:, njj:njj + 1])
                rs = stat.tile([128, 1], F32, tag="rs", bufs=4)
                nc.vector.reduce_sum(out=rs[:, :], in_=s8[:, :], axis=AX)
                nc.vector.reciprocal(out=rs[:, :], in_=rs[:, :])
                # x5 rows for this mi, scaled by rss*SCALE -> fp8
                nc.vector.tensor_scalar(
                    out=x5t8[pr][:, sub, :], in0=x5t2[pr][:, sub, :],
                    scalar1=rs[:, :], scalar2=SCALE,
                    op0=OP.mult, op1=OP.mult)

            # ---------- P partial (fp8 DoubleRow) + chunked ReduceScatter ---
            for njj in range(8):
                for mc in range(4):
                    pt = ps.tile([128, 512], F32, tag="ps")
                    for p in range(2):
                        nc.tensor.matmul(
                            pt[:, :],
                            x5t8[p][:, :, mc * 128:(mc + 1) * 128],
                            A8[p][:, :, njj * 512:(njj + 1) * 512],
                            start=(p == 0), stop=(p == 1),
                            perf_mode=PM.DoubleRow)
                    st = stg.tile([128, 512], BF16, tag="pstg")
                    nc.vector.tensor_copy(out=st[:, :], in_=pt[:, :])
                    cch = min(njj // 2, 2)
                    cbase = [0, 1024, 2048, 2048][njj // 2]
                    nc.sync.dma_start(
                        out=p_drams[cch][mc * 128:(mc + 1) * 128,
                                         njj * 512 - cbase:
                                         njj * 512 - cbase + 512],
                        in_=st[:, :])
                if njj in (1, 3, 7):
                    cch = min(njj // 2, 2)
                    nc.gpsimd.collective_compute(
                        "ReduceScatter", OP.add, replica_groups=groups,
                        ins=[p_drams[cch][:, :]], outs=[p_rss[cch][:, :]])

            # ---------- c3 = conv3x3(x) stride2 -> (CH, 1024) bf16 ----------
            c3 = sm.tile([CH, PIX], BF16, tag="c3")
            for ohc in range(2):
                o0 = ohc * 16
                pt = ps.tile([128, 512], F32, tag="ps")
                first = True
                for ti, (kh, kw) in enumerate(taps):
                    oo0 = o0
                    ih0 = 2 * oo0 - 1 + kh
                    if ih0 < 0:
                        oo0 += 1
                        ih0 += 2
                    cnt_oh = (o0 + 16) - oo0
                    if kw < 1:
                        iw0, ow0, cnt_ow = 1, 1, 31
                    else:
                        iw0, ow0, cnt_ow = kw - 1, 0, 32
                    for k in range(4):
                        src = xsb[k]
                        rhs = bass.AP(
                            tensor=src.tensor,
                            offset=src.offset + ih0 * 64 + iw0,
                            ap=[list(src.ap[0]),
                                [128, cnt_oh], [2, cnt_ow]])
                        outv = pt[:CH, :].rearrange(
                            "p (a b) -> p a b", a=16)[
                            :, oo0 - o0:oo0 - o0 + cnt_oh,
                            ow0:ow0 + cnt_ow]
                        nc.tensor.matmul(
                            outv, w3sb[k][:, kh * 3 + kw, :], rhs,
                            start=first,
                            stop=(ti == len(taps) - 1 and k == 3))
                        first = False
                nc.vector.tensor_scalar_add(
                    out=c3[:, ohc * 512:(ohc + 1) * 512], in0=pt[:CH, :],
                    scalar1=b3_sb[:, :])

        with tc.tile_pool(name="psC", bufs=3, space="PSUM") as psC, \
             tc.tile_pool(name="psS", bufs=2, space="PSUM") as psS:
            # x4_3 = sigmoid(leaky_relu(c3)); stream transposes
            x43 = sm.tile([CH, PIX], BF16, tag="x43")
            nc.scalar.activation(out=x43[:, :], in_=c3[:, :], func=AF.Lrelu,
                                 alpha=0.2)
            nc.scalar.activation(out=x43[:, :], in_=x43[:, :], func=AF.Sigmoid)
            tc3 = sm.tile([CH, PIX], BF16, tag="tc3")
            nc.vector.transpose(out=tc3[:, :], in_=c3[:, :])
            tx43 = sm.tile([CH, PIX], BF16, tag="tx43")
            nc.vector.transpose(out=tx43[:, :], in_=x43[:, :])

            def tview(t, clo):
                # lhsT/rhs view for channel (chi, clo): (32 w, 32 a@stride32)
                return t.rearrange("p (a c) -> p a c", a=H2)[:, :, clo]

            def product(ta, tb, dst_sm, emul):
                """S2[chi*32+a, clo*32+b] -> softmax over b -> dst_sm."""
                pS = psS.tile([CH, PIX], F32, tag="pS")
                for c in range(CH):
                    chi, clo = divmod(c, H2)
                    sl = slice(chi * H2, (chi + 1) * H2)
                    nc.tensor.matmul(
                        pS[sl, clo * H2:(clo + 1) * H2],
                        tview(ta[sl, :], clo), tview(tb[sl, :], clo),
                        start=True, stop=True)
                # softmax over b (free innermost 32), no max-sub (range safe)
                ssum = stat.tile([CH, H2], F32, tag="ssum")
                nc.scalar.activation(out=dst_sm[:, :], in_=pS[:, :],
                                     func=AF.Exp)
                dv = dst_sm.rearrange("p (c b) -> p c b", c=H2)
                nc.vector.reduce_sum(out=_unit(ssum[:, :]), in_=dv, axis=AX)
                nc.vector.reciprocal(out=ssum[:, :], in_=ssum[:, :])
                emul.tensor_tensor(out=dv, in0=dv,
                                   in1=_bcast(ssum[:, :], 1, H2),
                                   op=OP.mult)

            # x3_2 = softmax(c3 @ x43^T)  (overlaps the RS1 window)
            x32 = sm.tile([CH, PIX], BF16, tag="x32")
            product(tc3, tx43, x32, nc.vector)
            tx32 = sm.tile([CH, PIX], BF16, tag="tx32")
            nc.vector.transpose(out=tx32[:, :], in_=x32[:, :])

            # ---------- x6 softmax + x7 + x ----------
            p6 = big.tile([CH, HW], BF16, tag="p6")
            s6c = stat.tile([CH, 3], F32, tag="s6c")
            for cch, (j0, j1) in enumerate(
                    [(0, 1024), (1024, 2048), (2048, 4096)]):
                nc.sync.dma_start(out=p6[:, j0:j1], in_=p_rss[cch][:, :])
                nc.scalar.activation(
                    out=p6[:, j0:j1], in_=p6[:, j0:j1],
                    func=AF.Exp, scale=1.0 / SCALE,
                    accum_out=s6c[:, cch:cch + 1])
            r6 = stat.tile([CH, 1], F32, tag="r6")
            nc.vector.reduce_sum(out=r6[:, :], in_=s6c[:, :], axis=AX)
            nc.vector.reciprocal(out=r6[:, :], in_=r6[:, :])
            # z / final softmax over W, in two h-halves so the x1_2 conv
            # can start on the first half early (no max-sub: |z| <= ~6)
            z = big.tile([CH, HW], F32, tag="z")
            zv = z.rearrange("p (h w) -> p h w", h=H)
            zs = stat.tile([CH, H], F32, tag="zs")
            x11 = big.tile([CH, HW], BF16, tag="x11")
            xv11 = x11.rearrange("p (h w) -> p h w", h=H)
            for hh in range(2):
                cs = slice(hh * 2048, (hh + 1) * 2048)
                hs = slice(hh * 32, (hh + 1) * 32)
                nc.vector.scalar_tensor_tensor(
                    out=z[:, cs], in0=p6[:, cs], scalar=r6[:, :],
                    in1=xblk_sb[:, cs], op0=OP.mult, op1=OP.add)
                nc.scalar.activation(out=z[:, cs], in_=z[:, cs], func=AF.Exp)
                nc.vector.reduce_sum(out=_unit(zs[:, hs]), in_=zv[:, hs, :],
                                     axis=AX)
                nc.vector.reciprocal(out=zs[:, hs], in_=zs[:, hs])
                nc.vector.tensor_tensor(
                    out=xv11[:, hs, :], in0=zv[:, hs, :],
                    in1=_bcast(zs[:, hs], 1, W), op=OP.mult)

            # ---------- x1_2 partial conv + chunked ReduceScatter ----------
            for ohc in range(2):
                o0 = ohc * 16
                for mc in range(4):
                    pt = psC.tile([128, 512], F32, tag="psC")
                    first = True
                    for ti, (kh, kw) in enumerate(taps):
                        oo0 = o0
                        ih0 = 2 * oo0 - 1 + kh
                        if ih0 < 0:
                            oo0 += 1
                            ih0 += 2
                        cnt_oh = (o0 + 16) - oo0
                        if kw < 1:
                            iw0, ow0, cnt_ow = 1, 1, 31
                        else:
                            iw0, ow0, cnt_ow = kw - 1, 0, 32
                        rhs = bass.AP(
                            tensor=x11.tensor,
                            offset=x11.offset + ih0 * 64 + iw0,
                            ap=[list(x11.ap[0]),
                                [128, cnt_oh], [2, cnt_ow]])
                        outv = pt[:, :].rearrange(
                            "p (a b) -> p a b", a=16)[
                            :, oo0 - o0:oo0 - o0 + cnt_oh,
                            ow0:ow0 + cnt_ow]
                        nc.tensor.matmul(
                            outv,
                            w3b_sb[:, kh * 3 + kw,
                                   mc * 128:(mc + 1) * 128],
                            rhs, start=first, stop=(ti == len(taps) - 1))
                        first = False
                    st = stg.tile([128, 512], BF16, tag="x12stg", bufs=2)
                    nc.vector.tensor_copy(out=st[:, :], in_=pt[:, :])
                    nc.sync.dma_start(
                        out=x12ds[ohc][mc * 128:(mc + 1) * 128, :],
                        in_=st[:, :])
                nc.gpsimd.collective_compute(
                    "ReduceScatter", OP.add, replica_groups=groups,
                    ins=[x12ds[ohc][:, :]], outs=[x12rss[ohc][:, :]])

            # ---------- per-channel products on PE ----------
            x12 = sm.tile([CH, PIX], BF16, tag="x12")
            for ohc in range(2):
                nc.sync.dma_start(out=x12[:, ohc * 512:(ohc + 1) * 512],
                                  in_=x12rss[ohc][:, :])
            nc.vector.tensor_scalar_add(out=x12[:, :], in0=x12[:, :],
                                        scalar1=b3_sb[:, :])
            tx12 = sm.tile([CH, PIX], BF16, tag="tx12")
            nc.vector.transpose(out=tx12[:, :], in_=x12[:, :])

            # x2_2 = softmax(x1_2 @ c3^T);  x3_3 = softmax(x1_2 @ x3_2^T)
            x22 = sm.tile([CH, PIX], BF16, tag="x22")
            product(tx12, tc3, x22, nc.vector)
            x33 = sm.tile([CH, PIX], BF16, tag="x33")
            product(tx12, tx32, x33, nc.vector)

            # ---------- x_f = relu(x3_3 + x2_2 + c3), back to c-layout ------
            nc.vector.tensor_tensor(out=x22[:, :], in0=x22[:, :],
                                    in1=x33[:, :], op=OP.add)
            # bounce through DRAM: write (chi,a),(clo,b) -> (c,(a,b)) order
            xfT = sm.tile([CH, PIX], BF16, tag="xfT")
            for chi in range(2):
                dst = bass.AP(
                    tensor=bT_dram, offset=chi * H2 * PIX,
                    ap=[[H2, H2], [PIX, H2], [1, H2]])  # (a-part, clo, b)
                nc.sync.dma_start(
                    out=dst,
                    in_=x22[chi * H2:(chi + 1) * H2, :])
            nc.sync.dma_start(out=xfT[:, :], in_=bT_dram[:, :])
            xf = sm.tile([CH, PIX], BF16, tag="xf")
            nc.vector.tensor_tensor(out=xf[:, :], in0=xfT[:, :], in1=c3[:, :],
                                    op=OP.add)
            nc.scalar.activation(out=xf[:, :], in_=xf[:, :], func=AF.Relu)

            # ---------- bilinear 2x upsample (half-pixel centers) ----------
            uh = big.tile([CH, H * W2], BF16, tag="uh")      # (CH, 64, 32)
            xv = xf.rearrange("p (h w) -> p h w", h=H2)
            uv = uh.rearrange("p (h w) -> p h w", h=H)
            nc.vector.tensor_copy(out=uv[:, 0, :], in_=xv[:, 0, :])
            nc.vector.tensor_copy(out=uv[:, H - 1, :], in_=xv[:, H2 - 1, :])
            dif = sm.tile([CH, (H2 - 1) * W2], BF16, tag="dif")
            dv = dif.rearrange("p (h w) -> p h w", h=H2 - 1)
            nc.vector.tensor_tensor(out=dv, in0=xv[:, 0:H2 - 1, :],
                                    in1=xv[:, 1:H2, :], op=OP.subtract)
            ev = bass.AP(tensor=uh.tensor, offset=uh.offset + 2 * W2,
                         ap=[list(uh.ap[0]), [2 * W2, H2 - 1], [1, W2]])
            nc.vector.scalar_tensor_tensor(
                out=ev, in0=dv, scalar=0.25, in1=xv[:, 1:H2, :],
                op0=OP.mult, op1=OP.add)
            ov = bass.AP(tensor=uh.tensor, offset=uh.offset + W2,
                         ap=[list(uh.ap[0]), [2 * W2, H2 - 1], [1, W2]])
            nc.vector.scalar_tensor_tensor(
                out=ov, in0=dv, scalar=-0.25, in1=xv[:, 0:H2 - 1, :],
                op0=OP.mult, op1=OP.add)
            # cols (w)
            outsb = big.tile([CH, HW], BF16, tag="outsb")
            ov2 = outsb.rearrange("p (h w) -> p h w", h=H)
            uv2 = uh.rearrange("p (h w) -> p h w", h=H)
            nc.vector.tensor_copy(out=ov2[:, :, 0], in_=uv2[:, :, 0])
            nc.vector.tensor_copy(out=ov2[:, :, W - 1], in_=uv2[:, :, W2 - 1])
            difw = sm.tile([CH, H * (W2 - 1)], BF16, tag="difw")
            dwv = difw.rearrange("p (h w) -> p h w", h=H)
            nc.vector.tensor_tensor(out=dwv, in0=uv2[:, :, 0:W2 - 1],
                                    in1=uv2[:, :, 1:W2], op=OP.subtract)
            evw = bass.AP(tensor=outsb.tensor, offset=outsb.offset + 2,
                          ap=[list(outsb.ap[0]), [W, H], [2, W2 - 1]])
            nc.vector.scalar_tensor_tensor(
                out=evw, in0=dwv, scalar=0.25, in1=uv2[:, :, 1:W2],
                op0=OP.mult, op1=OP.add)
            ovw = bass.AP(tensor=outsb.tensor, offset=outsb.offset + 1,
                          ap=[list(outsb.ap[0]), [W, H], [2, W2 - 1]])
            nc.vector.scalar_tensor_tensor(
                out=ovw, in0=dwv, scalar=-0.25, in1=uv2[:, :, 0:W2 - 1],
                op0=OP.mult, op1=OP.add)

            nc.sync.dma_start(out=out_ext[:, :], in_=outsb[:, :])

    return nc


_NC_CACHE = {}
_LAST_IN_MAPS = None


def kernel(x, w_pos, b_pos, w3, b3):
    x = np.asarray(x, np.float32)
    w_pos = np.asarray(w_pos, np.float32)
    b_pos = np.asarray(b_pos, np.float32)
    w3 = np.asarray(w3, np.float32)
    b3 = np.asarray(b3, np.float32)

    x_ = x[0]                                   # (C, H, W)
    xhw = x_.reshape(C, HW)                     # i = h*W + w
    xwh = x_.transpose(0, 2, 1).reshape(C, HW)  # i = w*H + h
    bf = lambda a: np.ascontiguousarray(a).astype(_BF)  # noqa: E731

    xhw_bf = bf(xhw)
    wposT = bf(w_pos.reshape(C8, C).T)
    bpos = np.ascontiguousarray(b_pos.reshape(C8, 1))
    w3b_all = bf(w3.transpose(1, 2, 3, 0).reshape(C, 9, C))  # (cin, tap, cout)

    in_maps = []
    for m in range(NCORE):
        w3s = w3[m * CH:(m + 1) * CH]           # (CH, C, 3, 3)
        w3t = w3s.transpose(1, 2, 3, 0).reshape(C, 9, CH).reshape(4, 128, 9,
                                                                  CH)
        x5T = xhw[:, m * IB:(m + 1) * IB].T     # (IB, C)
        x5T2 = x5T.reshape(2, 2, 128, C).transpose(0, 2, 1, 3)
        in_maps.append({
            "xhw": xhw_bf,
            "xP": bf(xwh[:, m * IB:(m + 1) * IB]),
            "x5T2": bf(x5T2),
            "xblk": np.ascontiguousarray(xhw[m * CH:(m + 1) * CH, :]),
            "wposT": wposT,
            "bpos": bpos,
            "w3a": bf(w3t),
            "w3b": np.ascontiguousarray(
                w3b_all[m * CH:(m + 1) * CH]),
            "b3": np.ascontiguousarray(b3[m * CH:(m + 1) * CH].reshape(CH,
                                                                       1)),
        })

    global _LAST_IN_MAPS
    _LAST_IN_MAPS = in_maps
    if "nc" not in _NC_CACHE:
        nc_ = _build_real()
        nc_.finalize()
        _NC_CACHE["nc"] = nc_
    nc = _NC_CACHE["nc"]

    res = run_bass_kernel_spmd(nc, in_maps, core_ids=list(range(NCORE)))
    outs = [np.asarray(res.results[m]["out"], np.float32)
            for m in range(NCORE)]
    full = np.concatenate(outs, axis=0).reshape(1, C, H, W)
    return full


# revision 18
# speedup vs baseline: 1.0001x; 1.0001x over previous
import sys

import numpy as np

sys.path.insert(0, "/opt/trn_rl_repo")

import ml_dtypes  # noqa: E402

import concourse.bacc as bacc  # noqa: E402
import concourse.bass as bass  # noqa: E402
import concourse.tile as tile  # noqa: E402
from concourse import masks, mybir  # noqa: E402
from concourse.bass_utils import run_bass_kernel_spmd  # noqa: E402

C, H, W = 512, 64, 64
HW = H * W          # 4096
C8 = 64             # pos-att channels
NCORE = 8
IB = HW // NCORE    # 512 spatial rows of A per core
CH = C // NCORE     # 64 channels per core
H2 = W2 = 32
PIX = H2 * W2       # 1024
SCALE = 32.0        # fp8 range scaling for the P matmul
F32 = mybir.dt.float32
BF16 = mybir.dt.bfloat16
FP8 = mybir.dt.float8e4
AX = mybir.AxisListType.X
OP = mybir.AluOpType
AF = mybir.ActivationFunctionType
PM = mybir.MatmulPerfMode

_BF = ml_dtypes.bfloat16


def _bcast(ap, pos, n):
    """Insert a stride-0 (broadcast) free dim of size n at free position pos."""
    a = [list(d) for d in ap.ap]
    a.insert(1 + pos, [0, n])
    return bass.AP(tensor=ap.tensor, offset=ap.offset, ap=a)


def _unit(ap):
    """Append a trailing unit free dim (for reduce outputs)."""
    a = [list(d) for d in ap.ap] + [[0, 1]]
    return bass.AP(tensor=ap.tensor, offset=ap.offset, ap=a)


def _build_real():
    nc = bacc.Bacc()

    xhw = nc.declare_dram_parameter("xhw", [C, HW], BF16, isOutput=False)
    xP = nc.declare_dram_parameter("xP", [C, IB], BF16, isOutput=False)
    x5T2 = nc.declare_dram_parameter("x5T2", [2, 128, 2, C], BF16,
                                     isOutput=False)
    xblk = nc.declare_dram_parameter("xblk", [CH, HW], F32, isOutput=False)
    wposT = nc.declare_dram_parameter("wposT", [C, C8], BF16, isOutput=False)
    bpos = nc.declare_dram_parameter("bpos", [C8, 1], F32, isOutput=False)
    w3a = nc.declare_dram_parameter("w3a", [4, 128, 9, CH], BF16,
                                    isOutput=False)
    w3b = nc.declare_dram_parameter("w3b", [CH, 9, C], BF16, isOutput=False)
    b3 = nc.declare_dram_parameter("b3", [CH, 1], F32, isOutput=False)
    out_ext = nc.declare_dram_parameter("out", [CH, HW], BF16, isOutput=True)

    warm_in = nc.dram_tensor("warm_in", [NCORE, 64], BF16)
    warm_out = nc.dram_tensor("warm_out", [1, 64], BF16)
    p_drams = []
    p_rss = []
    for i, sz in enumerate([2048, 1024, 1024]):
        pd = nc.dram_tensor(f"p_bounce{i}", [C, sz], BF16)
        p_drams.append(pd)
        pr = nc.dram_tensor(f"p_rs{i}", [CH, sz], BF16)
        p_rss.append(pr)
    x12ds = []
    x12rss = []
    for i in range(2):
        xd = nc.dram_tensor(f"x12_bounce{i}", [C, 512], BF16)
        x12ds.append(xd)
        xr = nc.dram_tensor(f"x12_rs{i}", [CH, 512], BF16)
        x12rss.append(xr)
    bT_dram = nc.dram_tensor("bT_dram", [CH, PIX], BF16)

    groups = [list(range(NCORE))]
    taps = [(1, 1)] + [(kh, kw) for kh in range(3) for kw in range(3)
                       if (kh, kw) != (1, 1)]

    with tile.TileContext(nc) as tc, \
         tc.tile_pool(name="big", bufs=1) as big, \
         tc.tile_pool(name="sm", bufs=1) as sm, \
         tc.tile_pool(name="stg", bufs=4) as stg, \
         tc.tile_pool(name="stat", bufs=2) as stat:

        # Warm-up collective with no data deps: posts immediately, absorbs
        # the comm-init barrier + inter-core launch skew off the critical
        # path (the first real collective otherwise pays it).
        nc.gpsimd.collective_compute(
            "ReduceScatter", OP.add, replica_groups=groups,
            ins=[warm_in[:, :]], outs=[warm_out[:, :]])

        # ---------- loads (in consumption order) ----------
        wp = []
        xsb = []
        for k in range(4):
            t = sm.tile([128, C8], BF16, tag=f"wp{k}")
            nc.sync.dma_start(out=t[:, :], in_=wposT[k * 128:(k + 1) * 128, :])
            wp.append(t)
            t = big.tile([128, HW], BF16, tag=f"xsb{k}")
            nc.sync.dma_start(out=t[:, :2048],
                              in_=xhw[k * 128:(k + 1) * 128, :2048])
            nc.sync.dma_start(out=t[:, 2048:],
                              in_=xhw[k * 128:(k + 1) * 128, 2048:])
            xsb.append(t)
        bpos_sb = sm.tile([C8, 1], F32, tag="bpos")
        nc.sync.dma_start(out=bpos_sb[:, :], in_=bpos[:, :])
        xp = []
        for k in range(4):
            t = sm.tile([128, IB], BF16, tag=f"xp{k}")
            nc.sync.dma_start(out=t[:, :], in_=xP[k * 128:(k + 1) * 128, :])
            xp.append(t)
        x5t2 = []
        for p in range(2):
            t = sm.tile([128, 2, C], BF16, tag=f"x5t2_{p}")
            nc.sync.dma_start(out=t[:, :, :], in_=x5T2[p, :, :, :])
            x5t2.append(t)
        w3sb = []
        for k in range(4):
            t = sm.tile([128, 9, CH], BF16, tag=f"w3a{k}")
            nc.sync.dma_start(out=t[:, :, :], in_=w3a[k, :, :, :])
            w3sb.append(t)
        b3_sb = sm.tile([CH, 1], F32, tag="b3")
        nc.sync.dma_start(out=b3_sb[:, :], in_=b3[:, :])
        xblk_sb = big.tile([CH, HW], F32, tag="xblk")
        nc.sync.dma_start(out=xblk_sb[:, :], in_=xblk[:, :])
        w3b_sb = sm.tile([CH, 9, C], BF16, tag="w3b")
        nc.sync.dma_start(out=w3b_sb[:, :, :], in_=w3b[:, :, :])

        A8 = []
        x5t8 = []
        for p in range(2):
            a8t = big.tile([128, 2, HW], FP8, tag=f"A8_{p}")
            A8.append(a8t)
            x58t = sm.tile([128, 2, C], FP8, tag=f"x5t8_{p}")
            x5t8.append(x58t)

        with tc.tile_pool(name="ps8", bufs=8, space="PSUM") as ps:
            # ---------- x3f = w_pos @ x_hw + b  (C8, HW), bf16 ----------
            x3f = big.tile([C8, HW], BF16, tag="x3f")
            for njj in range(8):
                pt = ps.tile([128, 512], F32, tag="ps")
                for k in range(4):
                    nc.tensor.matmul(
                        pt[:C8, :], wp[k][:, :],
                        xsb[k][:, njj * 512:(njj + 1) * 512],
                        start=(k == 0), stop=(k == 3))
                nc.vector.tensor_scalar_add(
                    out=x3f[:, njj * 512:(njj + 1) * 512], in0=pt[:C8, :],
                    scalar1=bpos_sb[:, :])

            # ---------- x3b = w_pos @ xP + b  (C8, IB) ----------
            x3b = sm.tile([C8, IB], BF16, tag="x3b")
            pt = ps.tile([128, 512], F32, tag="ps")
            for k in range(4):
                nc.tensor.matmul(
                    pt[:C8, :], wp[k][:, :], xp[k][:, :],
                    start=(k == 0), stop=(k == 3))
            nc.vector.tensor_scalar_add(
                out=x3b[:, :], in0=pt[:C8, :], scalar1=bpos_sb[:, :])

            # ---------- A rows + softmax -> A8 (fp8), x5 scale ----------
            for mi in range(4):
                pr, sub = divmod(mi, 2)
                mx8 = stat.tile([128, 8], F32, tag="mx8")
                pts = []
                for njj in range(8):
                    pt = ps.tile([128, 512], F32, tag="ps")
                    nc.tensor.matmul(
                        pt[:, :], x3b[:, mi * 128:(mi + 1) * 128],
                        x3f[:, njj * 512:(njj + 1) * 512],
                        start=True, stop=True)
                    nc.vector.reduce_max(
                        out=mx8[:, njj:njj + 1], in_=pt[:, :], axis=AX)
                    pts.append(pt)
                mxn = stat.tile([128, 1], F32, tag="mxn")
                nc.vector.reduce_max(out=mxn[:, :], in_=mx8[:, :], axis=AX)
                nc.vector.tensor_scalar_mul(out=mxn[:, :], in0=mxn[:, :],
                                            scalar1=-1.0)
                s8 = stat.tile([128, 8], F32, tag="s8")
                for njj in range(8):
                    nc.scalar.activation(
                        out=A8[pr][:, sub, njj * 512:(njj + 1) * 512],
                        in_=pts[njj][:, :],
                        func=AF.Exp, bias=mxn[:, :], scale=1.0,
                        accum_out=s8[:, njj:njj + 1])
                rs = stat.tile([128, 1], F32, tag="rs", bufs=4)
                nc.vector.reduce_sum(out=rs[:, :], in_=s8[:, :], axis=AX)
                nc.vector.reciprocal(out=rs[:, :], in_=rs[:, :])
                # x5 rows for this mi, scaled by rss*SCALE -> fp8
                nc.vector.tensor_scalar(
                    out=x5t8[pr][:, sub, :], in0=x5t2[pr][:, sub, :],
                    scalar1=rs[:, :], scalar2=SCALE,
                    op0=OP.mult, op1=OP.mult)

            # ---------- P partial (fp8 DoubleRow) + chunked ReduceScatter ---
            for njj in range(8):
                for mc in range(4):
                    pt = ps.tile([128, 512], F32, tag="ps")
                    for p in range(2):
                        nc.tensor.matmul(
                            pt[:, :],
                            x5t8[p][:, :, mc * 128:(mc + 1) * 128],
                            A8[p][:, :, njj * 512:(njj + 1) * 512],
                            start=(p == 0), stop=(p == 1),
                            perf_mode=PM.DoubleRow)
                    st = stg.tile([128, 512], BF16, tag="pstg")
                    nc.vector.tensor_copy(out=st[:, :], in_=pt[:, :])
                    cch = max(njj // 2 - 1, 0)
                    cbase = [0, 0, 2048, 3072][njj // 2]
                    nc.sync.dma_start(
                        out=p_drams[cch][mc * 128:(mc + 1) * 128,
                                         njj * 512 - cbase:
                                         njj * 512 - cbase + 512],
                        in_=st[:, :])
                if njj in (3, 5, 7):
                    cch = max(njj // 2 - 1, 0)
                    nc.gpsimd.collective_compute(
                        "ReduceScatter", OP.add, replica_groups=groups,
                        ins=[p_drams[cch][:, :]], outs=[p_rss[cch][:, :]])

            # ---------- c3 = conv3x3(x) stride2 -> (CH, 1024) bf16 ----------
            c3 = sm.tile([CH, PIX], BF16, tag="c3")
            for ohc in range(2):
                o0 = ohc * 16
                pt = ps.tile([128, 512], F32, tag="ps")
                first = True
                for ti, (kh, kw) in enumerate(taps):
                    oo0 = o0
                    ih0 = 2 * oo0 - 1 + kh
                    if ih0 < 0:
                        oo0 += 1
                        ih0 += 2
                    cnt_oh = (o0 + 16) - oo0
                    if kw < 1:
                        iw0, ow0, cnt_ow = 1, 1, 31
                    else:
                        iw0, ow0, cnt_ow = kw - 1, 0, 32
                    for k in range(4):
                        src = xsb[k]
                        rhs = bass.AP(
                            tensor=src.tensor,
                            offset=src.offset + ih0 * 64 + iw0,
                            ap=[list(src.ap[0]),
                                [128, cnt_oh], [2, cnt_ow]])
                        outv = pt[:CH, :].rearrange(
                            "p (a b) -> p a b", a=16)[
                            :, oo0 - o0:oo0 - o0 + cnt_oh,
                            ow0:ow0 + cnt_ow]
                        nc.tensor.matmul(
                            outv, w3sb[k][:, kh * 3 + kw, :], rhs,
                            start=first,
                            stop=(ti == len(taps) - 1 and k == 3))
                        first = False
                nc.vector.tensor_scalar_add(
                    out=c3[:, ohc * 512:(ohc + 1) * 512], in0=pt[:CH, :],
                    scalar1=b3_sb[:, :])

        with tc.tile_pool(name="psC", bufs=3, space="PSUM") as psC, \
             tc.tile_pool(name="psS", bufs=2, space="PSUM") as psS:
            # x4_3 = sigmoid(leaky_relu(c3)); stream transposes
            x43 = sm.tile([CH, PIX], BF16, tag="x43")
            nc.scalar.activation(out=x43[:, :], in_=c3[:, :], func=AF.Lrelu,
                                 alpha=0.2)
            nc.scalar.activation(out=x43[:, :], in_=x43[:, :], func=AF.Sigmoid)
            tc3 = sm.tile([CH, PIX], BF16, tag="tc3")
            nc.vector.transpose(out=tc3[:, :], in_=c3[:, :])
            tx43 = sm.tile([CH, PIX], BF16, tag="tx43")
            nc.vector.transpose(out=tx43[:, :], in_=x43[:, :])

            def tview(t, clo):
                # lhsT/rhs view for channel (chi, clo): (32 w, 32 a@stride32)
                return t.rearrange("p (a c) -> p a c", a=H2)[:, :, clo]

            def product(ta, tb, dst_sm, emul):
                """S2[chi*32+a, clo*32+b] -> softmax over b -> dst_sm."""
                pS = psS.tile([CH, PIX], F32, tag="pS")
                for c in range(CH):
                    chi, clo = divmod(c, H2)
                    sl = slice(chi * H2, (chi + 1) * H2)
                    nc.tensor.matmul(
                        pS[sl, clo * H2:(clo + 1) * H2],
                        tview(ta[sl, :], clo), tview(tb[sl, :], clo),
                        start=True, stop=True)
                # softmax over b (free innermost 32), no max-sub (range safe)
                ssum = stat.tile([CH, H2], F32, tag="ssum")
                nc.scalar.activation(out=dst_sm[:, :], in_=pS[:, :],
                                     func=AF.Exp)
                dv = dst_sm.rearrange("p (c b) -> p c b", c=H2)
                nc.vector.reduce_sum(out=_unit(ssum[:, :]), in_=dv, axis=AX)
                nc.vector.reciprocal(out=ssum[:, :], in_=ssum[:, :])
                emul.tensor_tensor(out=dv, in0=dv,
                                   in1=_bcast(ssum[:, :], 1, H2),
                                   op=OP.mult)

            # x3_2 = softmax(c3 @ x43^T)  (overlaps the RS1 window)
            x32 = sm.tile([CH, PIX], BF16, tag="x32")
            product(tc3, tx43, x32, nc.vector)
            tx32 = sm.tile([CH, PIX], BF16, tag="tx32")
            nc.vector.transpose(out=tx32[:, :], in_=x32[:, :])

            # ---------- x6 softmax + x7 + x ----------
            p6 = big.tile([CH, HW], BF16, tag="p6")
            s6c = stat.tile([CH, 3], F32, tag="s6c")
            for cch, (j0, j1) in enumerate(
                    [(0, 2048), (2048, 3072), (3072, 4096)]):
                nc.sync.dma_start(out=p6[:, j0:j1], in_=p_rss[cch][:, :])
                nc.scalar.activation(
                    out=p6[:, j0:j1], in_=p6[:, j0:j1],
                    func=AF.Exp, scale=1.0 / SCALE,
                    accum_out=s6c[:, cch:cch + 1])
            r6 = stat.tile([CH, 1], F32, tag="r6")
            nc.vector.reduce_sum(out=r6[:, :], in_=s6c[:, :], axis=AX)
            nc.vector.reciprocal(out=r6[:, :], in_=r6[:, :])
            # z / final softmax over W, in two h-halves so the x1_2 conv
            # can start on the first half early (no max-sub: |z| <= ~6)
            z = big.tile([CH, HW], F32, tag="z")
            zv = z.rearrange("p (h w) -> p h w", h=H)
            zs = stat.tile([CH, H], F32, tag="zs")
            x11 = big.tile([CH, HW], BF16, tag="x11")
            xv11 = x11.rearrange("p (h w) -> p h w", h=H)
            for hh in range(2):
                cs = slice(hh * 2048, (hh + 1) * 2048)
                hs = slice(hh * 32, (hh + 1) * 32)
                nc.vector.scalar_tensor_tensor(
                    out=z[:, cs], in0=p6[:, cs], scalar=r6[:, :],
                    in1=xblk_sb[:, cs], op0=OP.mult, op1=OP.add)
                nc.scalar.activation(out=z[:, cs], in_=z[:, cs], func=AF.Exp)
                nc.vector.reduce_sum(out=_unit(zs[:, hs]), in_=zv[:, hs, :],
                                     axis=AX)
                nc.vector.reciprocal(out=zs[:, hs], in_=zs[:, hs])
                nc.vector.tensor_tensor(
                    out=xv11[:, hs, :], in0=zv[:, hs, :],
                    in1=_bcast(zs[:, hs], 1, W), op=OP.mult)

            # ---------- x1_2 partial conv + chunked ReduceScatter ----------
            for ohc in range(2):
                o0 = ohc * 16
                for mc in range(4):
                    pt = psC.tile([128, 512], F32, tag="psC")
                    first = True
                    for ti, (kh, kw) in enumerate(taps):
                        oo0 = o0
                        ih0 = 2 * oo0 - 1 + kh
                        if ih0 < 0:
                            oo0 += 1
                            ih0 += 2
                        cnt_oh = (o0 + 16) - oo0
                        if kw < 1:
                            iw0, ow0, cnt_ow = 1, 1, 31
                        else:
                            iw0, ow0, cnt_ow = kw - 1, 0, 32
                        rhs = bass.AP(
                            tensor=x11.tensor,
                            offset=x11.offset + ih0 * 64 + iw0,
                            ap=[list(x11.ap[0]),
                                [128, cnt_oh], [2, cnt_ow]])
                        outv = pt[:, :].rearrange(
                            "p (a b) -> p a b", a=16)[
                            :, oo0 - o0:oo0 - o0 + cnt_oh,
                            ow0:ow0 + cnt_ow]
                        nc.tensor.matmul(
                            outv,
                            w3b_sb[:, kh * 3 + kw,
                                   mc * 128:(mc + 1) * 128],
                            rhs, start=first, stop=(ti == len(taps) - 1))
                        first = False
                    st = stg.tile([128, 512], BF16, tag="x12stg", bufs=2)
                    nc.vector.tensor_copy(out=st[:, :], in_=pt[:, :])
                    nc.sync.dma_start(
                        out=x12ds[ohc][mc * 128:(mc + 1) * 128, :],
                        in_=st[:, :])
                nc.gpsimd.collective_compute(
                    "ReduceScatter", OP.add, replica_groups=groups,
                    ins=[x12ds[ohc][:, :]], outs=[x12rss[ohc][:, :]])

            # ---------- per-channel products on PE ----------
            x12 = sm.tile([CH, PIX], BF16, tag="x12")
            for ohc in range(2):
                nc.sync.dma_start(out=x12[:, ohc * 512:(ohc + 1) * 512],
                                  in_=x12rss[ohc][:, :])
            nc.vector.tensor_scalar_add(out=x12[:, :], in0=x12[:, :],
                                        scalar1=b3_sb[:, :])
            tx12 = sm.tile([CH, PIX], BF16, tag="tx12")
            nc.vector.transpose(out=tx12[:, :], in_=x12[:, :])

            # x2_2 = softmax(x1_2 @ c3^T);  x3_3 = softmax(x1_2 @ x3_2^T)
            x22 = sm.tile([CH, PIX], BF16, tag="x22")
            product(tx12, tc3, x22, nc.vector)
            x33 = sm.tile([CH, PIX], BF16, tag="x33")
            product(tx12, tx32, x33, nc.vector)

            # ---------- x_f = relu(x3_3 + x2_2 + c3), back to c-layout ------
            nc.vector.tensor_tensor(out=x22[:, :], in0=x22[:, :],
                                    in1=x33[:, :], op=OP.add)
            # bounce through DRAM: write (chi,a),(clo,b) -> (c,(a,b)) order
            xfT = sm.tile([CH, PIX], BF16, tag="xfT")
            for chi in range(2):
                dst = bass.AP(
                    tensor=bT_dram, offset=chi * H2 * PIX,
                    ap=[[H2, H2], [PIX, H2], [1, H2]])  # (a-part, clo, b)
                nc.sync.dma_start(
                    out=dst,
                    in_=x22[chi * H2:(chi + 1) * H2, :])
            nc.sync.dma_start(out=xfT[:, :], in_=bT_dram[:, :])
            xf = sm.tile([CH, PIX], BF16, tag="xf")
            nc.vector.tensor_tensor(out=xf[:, :], in0=xfT[:, :], in1=c3[:, :],
                                    op=OP.add)
            nc.scalar.activation(out=xf[:, :], in_=xf[:, :], func=AF.Relu)

            # ---------- bilinear 2x upsample (half-pixel centers) ----------
            uh = big.tile([CH, H * W2], BF16, tag="uh")      # (CH, 64, 32)
            xv = xf.rearrange("p (h w) -> p h w", h=H2)
            uv = uh.rearrange("p (h w) -> p h w", h=H)
            nc.vector.tensor_copy(out=uv[:, 0, :], in_=xv[:, 0, :])
            nc.vector.tensor_copy(out=uv[:, H - 1, :], in_=xv[:, H2 - 1, :])
            dif = sm.tile([CH, (H2 - 1) * W2], BF16, tag="dif")
            dv = dif.rearrange("p (h w) -> p h w", h=H2 - 1)
            nc.vector.tensor_tensor(out=dv, in0=xv[:, 0:H2 - 1, :],
                                    in1=xv[:, 1:H2, :], op=OP.subtract)
            ev = bass.AP(tensor=uh.tensor, offset=uh.offset + 2 * W2,
                         ap=[list(uh.ap[0]), [2 * W2, H2 - 1], [1, W2]])
            nc.vector.scalar_tensor_tensor(
                out=ev, in0=dv, scalar=0.25, in1=xv[:, 1:H2, :],
                op0=OP.mult, op1=OP.add)
            ov = bass.AP(tensor=uh.tensor, offset=uh.offset + W2,
                         ap=[list(uh.ap[0]), [2 * W2, H2 - 1], [1, W2]])
            nc.vector.scalar_tensor_tensor(
                out=ov, in0=dv, scalar=-0.25, in1=xv[:, 0:H2 - 1, :],
                op0=OP.mult, op1=OP.add)
            # cols (w)
            outsb = big.tile([CH, HW], BF16, tag="outsb")
            ov2 = outsb.rearrange("p (h w) -> p h w", h=H)
            uv2 = uh.rearrange("p (h w) -> p h w", h=H)
            nc.vector.tensor_copy(out=ov2[:, :, 0], in_=uv2[:, :, 0])
            nc.vector.tensor_copy(out=ov2[:, :, W - 1], in_=uv2[:, :, W2 - 1])
            difw = sm.tile([CH, H * (W2 - 1)], BF16, tag="difw")
            dwv = difw.rearrange("p (h w) -> p h w", h=H)
            nc.vector.tensor_tensor(out=dwv, in0=uv2[:, :, 0:W2 - 1],
                                    in1=uv2[:, :, 1:W2], op=OP.subtract)
            evw = bass.AP(tensor=outsb.tensor, offset=outsb.offset + 2,
                          ap=[list(outsb.ap[0]), [W, H], [2, W2 - 1]])
            nc.vector.scalar_tensor_tensor(
                out=evw, in0=dwv, scalar=0.25, in1=uv2[:, :, 1:W2],
                op0=OP.mult, op1=OP.add)
            ovw = bass.AP(tensor=outsb.tensor, offset=outsb.offset + 1,
                          ap=[list(outsb.ap[0]), [W, H], [2, W2 - 1]])
            nc.vector.scalar_tensor_tensor(
                out=ovw, in0=dwv, scalar=-0.25, in1=uv2[:, :, 0:W2 - 1],
                op0=OP.mult, op1=OP.add)

            nc.sync.dma_start(out=out_ext[:, :], in_=outsb[:, :])

    return nc


_NC_CACHE = {}
_LAST_IN_MAPS = None


def kernel(x, w_pos, b_pos, w3, b3):
    x = np.asarray(x, np.float32)
    w_pos = np.asarray(w_pos, np.float32)
    b_pos = np.asarray(b_pos, np.float32)
    w3 = np.asarray(w3, np.float32)
    b3 = np.asarray(b3, np.float32)

    x_ = x[0]                                   # (C, H, W)
    xhw = x_.reshape(C, HW)                     # i = h*W + w
    xwh = x_.transpose(0, 2, 1).reshape(C, HW)  # i = w*H + h
    bf = lambda a: np.ascontiguousarray(a).astype(_BF)  # noqa: E731

    xhw_bf = bf(xhw)
    wposT = bf(w_pos.reshape(C8, C).T)
    bpos = np.ascontiguousarray(b_pos.reshape(C8, 1))
    w3b_all = bf(w3.transpose(1, 2, 3, 0).reshape(C, 9, C))  # (cin, tap, cout)

    in_maps = []
    for m in range(NCORE):
        w3s = w3[m * CH:(m + 1) * CH]           # (CH, C, 3, 3)
        w3t = w3s.transpose(1, 2, 3, 0).reshape(C, 9, CH).reshape(4, 128, 9,
                                                                  CH)
        x5T = xhw[:, m * IB:(m + 1) * IB].T     # (IB, C)
        x5T2 = x5T.reshape(2, 2, 128, C).transpose(0, 2, 1, 3)
        in_maps.append({
            "xhw": xhw_bf,
            "xP": bf(xwh[:, m * IB:(m + 1) * IB]),
            "x5T2": bf(x5T2),
            "xblk": np.ascontiguousarray(xhw[m * CH:(m + 1) * CH, :]),
            "wposT": wposT,
            "bpos": bpos,
            "w3a": bf(w3t),
            "w3b": np.ascontiguousarray(
                w3b_all[m * CH:(m + 1) * CH]),
            "b3": np.ascontiguousarray(b3[m * CH:(m + 1) * CH].reshape(CH,
                                                                       1)),
        })

    global _LAST_IN_MAPS
    _LAST_IN_MAPS = in_maps
    if "nc" not in _NC_CACHE:
        nc_ = _build_real()
        nc_.finalize()
        _NC_CACHE["nc"] = nc_
    nc = _NC_CACHE["nc"]

    res = run_bass_kernel_spmd(nc, in_maps, core_ids=list(range(NCORE)))
    outs = [np.asarray(res.results[m]["out"], np.float32)
            for m in range(NCORE)]
    full = np.concatenate(outs, axis=0).reshape(1, C, H, W)
    return full


# revision 19
# speedup vs baseline: 1.0139x; 1.0138x over previous
import sys

import numpy as np

sys.path.insert(0, "/opt/trn_rl_repo")

import ml_dtypes  # noqa: E402

import concourse.bacc as bacc  # noqa: E402
import concourse.bass as bass  # noqa: E402
import concourse.tile as tile  # noqa: E402
from concourse import masks, mybir  # noqa: E402
from concourse.bass_utils import run_bass_kernel_spmd  # noqa: E402

C, H, W = 512, 64, 64
HW = H * W          # 4096
C8 = 64             # pos-att channels
NCORE = 8
IB = HW // NCORE    # 512 spatial rows of A per core
CH = C // NCORE     # 64 channels per core
H2 = W2 = 32
PIX = H2 * W2       # 1024
SCALE = 32.0        # fp8 range scaling for the P matmul
F32 = mybir.dt.float32
BF16 = mybir.dt.bfloat16
FP8 = mybir.dt.float8e4
AX = mybir.AxisListType.X
OP = mybir.AluOpType
AF = mybir.ActivationFunctionType
PM = mybir.MatmulPerfMode

_BF = ml_dtypes.bfloat16


def _bcast(ap, pos, n):
    """Insert a stride-0 (broadcast) free dim of size n at free position pos."""
    a = [list(d) for d in ap.ap]
    a.insert(1 + pos, [0, n])
    return bass.AP(tensor=ap.tensor, offset=ap.offset, ap=a)


def _unit(ap):
    """Append a trailing unit free dim (for reduce outputs)."""
    a = [list(d) for d in ap.ap] + [[0, 1]]
    return bass.AP(tensor=ap.tensor, offset=ap.offset, ap=a)


def _build_real():
    nc = bacc.Bacc()

    xhw = nc.declare_dram_parameter("xhw", [C, HW], BF16, isOutput=False)
    xP = nc.declare_dram_parameter("xP", [C, IB], BF16, isOutput=False)
    x5T2 = nc.declare_dram_parameter("x5T2", [2, 128, 2, C], BF16,
                                     isOutput=False)
    xblk = nc.declare_dram_parameter("xblk", [CH, HW], F32, isOutput=False)
    wposT = nc.declare_dram_parameter("wposT", [C, C8], BF16, isOutput=False)
    bpos = nc.declare_dram_parameter("bpos", [C8, 1], F32, isOutput=False)
    w3a = nc.declare_dram_parameter("w3a", [4, 128, 9, CH], BF16,
                                    isOutput=False)
    w3b = nc.declare_dram_parameter("w3b", [CH, 9, C], BF16, isOutput=False)
    b3 = nc.declare_dram_parameter("b3", [CH, 1], F32, isOutput=False)
    out_ext = nc.declare_dram_parameter("out", [CH, HW], BF16, isOutput=True)

    warm_in = nc.dram_tensor("warm_in", [NCORE, 64], BF16)
    warm_out = nc.dram_tensor("warm_out", [1, 64], BF16)
    p_drams = []
    p_rss = []
    for i, sz in enumerate([2048, 2048]):
        pd = nc.dram_tensor(f"p_bounce{i}", [C, sz], BF16)
        p_drams.append(pd)
        pr = nc.dram_tensor(f"p_rs{i}", [CH, sz], BF16)
        p_rss.append(pr)
    x12ds = []
    x12rss = []
    for i in range(2):
        xd = nc.dram_tensor(f"x12_bounce{i}", [C, 512], BF16)
        x12ds.append(xd)
        xr = nc.dram_tensor(f"x12_rs{i}", [CH, 512], BF16)
        x12rss.append(xr)
    bT_dram = nc.dram_tensor("bT_dram", [CH, PIX], BF16)

    groups = [list(range(NCORE))]
    taps = [(1, 1)] + [(kh, kw) for kh in range(3) for kw in range(3)
                       if (kh, kw) != (1, 1)]

    with tile.TileContext(nc) as tc, \
         tc.tile_pool(name="big", bufs=1) as big, \
         tc.tile_pool(name="sm", bufs=1) as sm, \
         tc.tile_pool(name="stg", bufs=4) as stg, \
         tc.tile_pool(name="stat", bufs=2) as stat:

        # Warm-up collective with no data deps: posts immediately, absorbs
        # the comm-init barrier + inter-core launch skew off the critical
        # path (the first real collective otherwise pays it).
        nc.gpsimd.collective_compute(
            "ReduceScatter", OP.add, replica_groups=groups,
            ins=[warm_in[:, :]], outs=[warm_out[:, :]])

        # ---------- loads (in consumption order) ----------
        wp = []
        xsb = []
        for k in range(4):
            t = sm.tile([128, C8], BF16, tag=f"wp{k}")
            nc.sync.dma_start(out=t[:, :], in_=wposT[k * 128:(k + 1) * 128, :])
            wp.append(t)
            t = big.tile([128, HW], BF16, tag=f"xsb{k}")
            nc.sync.dma_start(out=t[:, :2048],
                              in_=xhw[k * 128:(k + 1) * 128, :2048])
            nc.sync.dma_start(out=t[:, 2048:],
                              in_=xhw[k * 128:(k + 1) * 128, 2048:])
            xsb.append(t)
        bpos_sb = sm.tile([C8, 1], F32, tag="bpos")
        nc.sync.dma_start(out=bpos_sb[:, :], in_=bpos[:, :])
        xp = []
        for k in range(4):
            t = sm.tile([128, IB], BF16, tag=f"xp{k}")
            nc.sync.dma_start(out=t[:, :], in_=xP[k * 128:(k + 1) * 128, :])
            xp.append(t)
        x5t2 = []
        for p in range(2):
            t = sm.tile([128, 2, C], BF16, tag=f"x5t2_{p}")
            nc.sync.dma_start(out=t[:, :, :], in_=x5T2[p, :, :, :])
            x5t2.append(t)
        w3sb = []
        for k in range(4):
            t = sm.tile([128, 9, CH], BF16, tag=f"w3a{k}")
            nc.sync.dma_start(out=t[:, :, :], in_=w3a[k, :, :, :])
            w3sb.append(t)
        b3_sb = sm.tile([CH, 1], F32, tag="b3")
        nc.sync.dma_start(out=b3_sb[:, :], in_=b3[:, :])
        xblk_sb = big.tile([CH, HW], F32, tag="xblk")
        nc.sync.dma_start(out=xblk_sb[:, :], in_=xblk[:, :])
        w3b_sb = sm.tile([CH, 9, C], BF16, tag="w3b")
        nc.sync.dma_start(out=w3b_sb[:, :, :], in_=w3b[:, :, :])

        A8 = []
        x5t8 = []
        for p in range(2):
            a8t = big.tile([128, 2, HW], FP8, tag=f"A8_{p}")
            A8.append(a8t)
            x58t = sm.tile([128, 2, C], FP8, tag=f"x5t8_{p}")
            x5t8.append(x58t)

        with tc.tile_pool(name="ps8", bufs=8, space="PSUM") as ps:
            # ---------- x3f = w_pos @ x_hw + b  (C8, HW), bf16 ----------
            x3f = big.tile([C8, HW], BF16, tag="x3f")
            for njj in range(8):
                pt = ps.tile([128, 512], F32, tag="ps")
                for k in range(4):
                    nc.tensor.matmul(
                        pt[:C8, :], wp[k][:, :],
                        xsb[k][:, njj * 512:(njj + 1) * 512],
                        start=(k == 0), stop=(k == 3))
                nc.vector.tensor_scalar_add(
                    out=x3f[:, njj * 512:(njj + 1) * 512], in0=pt[:C8, :],
                    scalar1=bpos_sb[:, :])

            # ---------- x3b = w_pos @ xP + b  (C8, IB) ----------
            x3b = sm.tile([C8, IB], BF16, tag="x3b")
            pt = ps.tile([128, 512], F32, tag="ps")
            for k in range(4):
                nc.tensor.matmul(
                    pt[:C8, :], wp[k][:, :], xp[k][:, :],
                    start=(k == 0), stop=(k == 3))
            nc.vector.tensor_scalar_add(
                out=x3b[:, :], in0=pt[:C8, :], scalar1=bpos_sb[:, :])

            # ---------- A rows + softmax -> A8 (fp8), x5 scale ----------
            for mi in range(4):
                pr, sub = divmod(mi, 2)
                mx8 = stat.tile([128, 8], F32, tag="mx8")
                pts = []
                for njj in range(8):
                    pt = ps.tile([128, 512], F32, tag="ps")
                    nc.tensor.matmul(
                        pt[:, :], x3b[:, mi * 128:(mi + 1) * 128],
                        x3f[:, njj * 512:(njj + 1) * 512],
                        start=True, stop=True)
                    nc.vector.reduce_max(
                        out=mx8[:, njj:njj + 1], in_=pt[:, :], axis=AX)
                    pts.append(pt)
                mxn = stat.tile([128, 1], F32, tag="mxn")
                nc.vector.reduce_max(out=mxn[:, :], in_=mx8[:, :], axis=AX)
                nc.vector.tensor_scalar_mul(out=mxn[:, :], in0=mxn[:, :],
                                            scalar1=-1.0)
                s8 = stat.tile([128, 8], F32, tag="s8")
                for njj in range(8):
                    nc.scalar.activation(
                        out=A8[pr][:, sub, njj * 512:(njj + 1) * 512],
                        in_=pts[njj][:, :],
                        func=AF.Exp, bias=mxn[:, :], scale=1.0,
                        accum_out=s8[:, njj:njj + 1])
                rs = stat.tile([128, 1], F32, tag="rs", bufs=4)
                nc.vector.reduce_sum(out=rs[:, :], in_=s8[:, :], axis=AX)
                nc.vector.reciprocal(out=rs[:, :], in_=rs[:, :])
                # x5 rows for this mi, scaled by rss*SCALE -> fp8
                nc.vector.tensor_scalar(
                    out=x5t8[pr][:, sub, :], in0=x5t2[pr][:, sub, :],
                    scalar1=rs[:, :], scalar2=SCALE,
                    op0=OP.mult, op1=OP.mult)

            # ---------- P partial (fp8 DoubleRow) + chunked ReduceScatter ---
            for njj in range(8):
                for mc in range(4):
                    pt = ps.tile([128, 512], F32, tag="ps")
                    for p in range(2):
                        nc.tensor.matmul(
                            pt[:, :],
                            x5t8[p][:, :, mc * 128:(mc + 1) * 128],
                            A8[p][:, :, njj * 512:(njj + 1) * 512],
                            start=(p == 0), stop=(p == 1),
                            perf_mode=PM.DoubleRow)
                    st = stg.tile([128, 512], BF16, tag="pstg")
                    nc.vector.tensor_copy(out=st[:, :], in_=pt[:, :])
                    cch = njj // 4
                    cbase = (njj // 4) * 2048
                    nc.sync.dma_start(
                        out=p_drams[cch][mc * 128:(mc + 1) * 128,
                                         njj * 512 - cbase:
                                         njj * 512 - cbase + 512],
                        in_=st[:, :])
                if njj in (3, 7):
                    cch = njj // 4
                    nc.gpsimd.collective_compute(
                        "ReduceScatter", OP.add, replica_groups=groups,
                        ins=[p_drams[cch][:, :]], outs=[p_rss[cch][:, :]])

            # ---------- c3 = conv3x3(x) stride2 -> (CH, 1024) bf16 ----------
            c3 = sm.tile([CH, PIX], BF16, tag="c3")
            for ohc in range(2):
                o0 = ohc * 16
                pt = ps.tile([128, 512], F32, tag="ps")
                first = True
                for ti, (kh, kw) in enumerate(taps):
                    oo0 = o0
                    ih0 = 2 * oo0 - 1 + kh
                    if ih0 < 0:
                        oo0 += 1
                        ih0 += 2
                    cnt_oh = (o0 + 16) - oo0
                    if kw < 1:
                        iw0, ow0, cnt_ow = 1, 1, 31
                    else:
                        iw0, ow0, cnt_ow = kw - 1, 0, 32
                    for k in range(4):
                        src = xsb[k]
                        rhs = bass.AP(
                            tensor=src.tensor,
                            offset=src.offset + ih0 * 64 + iw0,
                            ap=[list(src.ap[0]),
                                [128, cnt_oh], [2, cnt_ow]])
                        outv = pt[:CH, :].rearrange(
                            "p (a b) -> p a b", a=16)[
                            :, oo0 - o0:oo0 - o0 + cnt_oh,
                            ow0:ow0 + cnt_ow]
                        nc.tensor.matmul(
                            outv, w3sb[k][:, kh * 3 + kw, :], rhs,
                            start=first,
                            stop=(ti == len(taps) - 1 and k == 3))
                        first = False
                nc.vector.tensor_scalar_add(
                    out=c3[:, ohc * 512:(ohc + 1) * 512], in0=pt[:CH, :],
                    scalar1=b3_sb[:, :])

        with tc.tile_pool(name="psC", bufs=3, space="PSUM") as psC, \
             tc.tile_pool(name="psS", bufs=2, space="PSUM") as psS:
            # x4_3 = sigmoid(leaky_relu(c3)); stream transposes
            x43 = sm.tile([CH, PIX], BF16, tag="x43")
            nc.scalar.activation(out=x43[:, :], in_=c3[:, :], func=AF.Lrelu,
                                 alpha=0.2)
            nc.scalar.activation(out=x43[:, :], in_=x43[:, :], func=AF.Sigmoid)
            tc3 = sm.tile([CH, PIX], BF16, tag="tc3")
            nc.vector.transpose(out=tc3[:, :], in_=c3[:, :])
            tx43 = sm.tile([CH, PIX], BF16, tag="tx43")
            nc.vector.transpose(out=tx43[:, :], in_=x43[:, :])

            def tview(t, clo):
                # lhsT/rhs view for channel (chi, clo): (32 w, 32 a@stride32)
                return t.rearrange("p (a c) -> p a c", a=H2)[:, :, clo]

            def product(ta, tb, dst_sm, emul):
                """S2[chi*32+a, clo*32+b] -> softmax over b -> dst_sm."""
                pS = psS.tile([CH, PIX], F32, tag="pS")
                for c in range(CH):
                    chi, clo = divmod(c, H2)
                    sl = slice(chi * H2, (chi + 1) * H2)
                    nc.tensor.matmul(
                        pS[sl, clo * H2:(clo + 1) * H2],
                        tview(ta[sl, :], clo), tview(tb[sl, :], clo),
                        start=True, stop=True)
                # softmax over b (free innermost 32), no max-sub (range safe)
                ssum = stat.tile([CH, H2], F32, tag="ssum")
                nc.scalar.activation(out=dst_sm[:, :], in_=pS[:, :],
                                     func=AF.Exp)
                dv = dst_sm.rearrange("p (c b) -> p c b", c=H2)
                nc.vector.reduce_sum(out=_unit(ssum[:, :]), in_=dv, axis=AX)
                nc.vector.reciprocal(out=ssum[:, :], in_=ssum[:, :])
                emul.tensor_tensor(out=dv, in0=dv,
                                   in1=_bcast(ssum[:, :], 1, H2),
                                   op=OP.mult)

            # x3_2 = softmax(c3 @ x43^T)  (overlaps the RS1 window)
            x32 = sm.tile([CH, PIX], BF16, tag="x32")
            product(tc3, tx43, x32, nc.vector)
            tx32 = sm.tile([CH, PIX], BF16, tag="tx32")
            nc.vector.transpose(out=tx32[:, :], in_=x32[:, :])

            # ---------- x6 softmax + x7 + x ----------
            p6 = big.tile([CH, HW], BF16, tag="p6")
            s6c = stat.tile([CH, 2], F32, tag="s6c")
            for cch, (j0, j1) in enumerate([(0, 2048), (2048, 4096)]):
                nc.sync.dma_start(out=p6[:, j0:j1], in_=p_rss[cch][:, :])
                nc.scalar.activation(
                    out=p6[:, j0:j1], in_=p6[:, j0:j1],
                    func=AF.Exp, scale=1.0 / SCALE,
                    accum_out=s6c[:, cch:cch + 1])
            r6 = stat.tile([CH, 1], F32, tag="r6")
            nc.vector.reduce_sum(out=r6[:, :], in_=s6c[:, :], axis=AX)
            nc.vector.reciprocal(out=r6[:, :], in_=r6[:, :])
            # z / final softmax over W, in two h-halves so the x1_2 conv
            # can start on the first half early (no max-sub: |z| <= ~6)
            z = big.tile([CH, HW], F32, tag="z")
            zv = z.rearrange("p (h w) -> p h w", h=H)
            zs = stat.tile([CH, H], F32, tag="zs")
            x11 = big.tile([CH, HW], BF16, tag="x11")
            xv11 = x11.rearrange("p (h w) -> p h w", h=H)
            for hh in range(2):
                cs = slice(hh * 2048, (hh + 1) * 2048)
                hs = slice(hh * 32, (hh + 1) * 32)
                nc.vector.scalar_tensor_tensor(
                    out=z[:, cs], in0=p6[:, cs], scalar=r6[:, :],
                    in1=xblk_sb[:, cs], op0=OP.mult, op1=OP.add)
                nc.scalar.activation(out=z[:, cs], in_=z[:, cs], func=AF.Exp)
                nc.vector.reduce_sum(out=_unit(zs[:, hs]), in_=zv[:, hs, :],
                                     axis=AX)
                nc.vector.reciprocal(out=zs[:, hs], in_=zs[:, hs])
                nc.vector.tensor_tensor(
                    out=xv11[:, hs, :], in0=zv[:, hs, :],
                    in1=_bcast(zs[:, hs], 1, W), op=OP.mult)

            # ---------- x1_2 partial conv + chunked ReduceScatter ----------
            for ohc in range(2):
                o0 = ohc * 16
                for mc in range(4):
                    pt = psC.tile([128, 512], F32, tag="psC")
                    first = True
                    for ti, (kh, kw) in enumerate(taps):
                        oo0 = o0
                        ih0 = 2 * oo0 - 1 + kh
                        if ih0 < 0:
                            oo0 += 1
                            ih0 += 2
                        cnt_oh = (o0 + 16) - oo0
                        if kw < 1:
                            iw0, ow0, cnt_ow = 1, 1, 31
                        else:
                            iw0, ow0, cnt_ow = kw - 1, 0, 32
                        rhs = bass.AP(
                            tensor=x11.tensor,
                            offset=x11.offset + ih0 * 64 + iw0,
                            ap=[list(x11.ap[0]),
                                [128, cnt_oh], [2, cnt_ow]])
                        outv = pt[:, :].rearrange(
                            "p (a b) -> p a b", a=16)[
                            :, oo0 - o0:oo0 - o0 + cnt_oh,
                            ow0:ow0 + cnt_ow]
                        nc.tensor.matmul(
                            outv,
                            w3b_sb[:, kh * 3 + kw,
                                   mc * 128:(mc + 1) * 128],
                            rhs, start=first, stop=(ti == len(taps) - 1))
                        first = False
                    st = stg.tile([128, 512], BF16, tag="x12stg", bufs=2)
                    nc.vector.tensor_copy(out=st[:, :], in_=pt[:, :])
                    nc.sync.dma_start(
                        out=x12ds[ohc][mc * 128:(mc + 1) * 128, :],
                        in_=st[:, :])
                nc.gpsimd.collective_compute(
                    "ReduceScatter", OP.add, replica_groups=groups,
                    ins=[x12ds[ohc][:, :]], outs=[x12rss[ohc][:, :]])

            # ---------- per-channel products on PE ----------
            x12 = sm.tile([CH, PIX], BF16, tag="x12")
            for ohc in range(2):
                nc.sync.dma_start(out=x12[:, ohc * 512:(ohc + 1) * 512],
                                  in_=x12rss[ohc][:, :])
            nc.vector.tensor_scalar_add(out=x12[:, :], in0=x12[:, :],
                                        scalar1=b3_sb[:, :])
            tx12 = sm.tile([CH, PIX], BF16, tag="tx12")
            nc.vector.transpose(out=tx12[:, :], in_=x12[:, :])

            # x2_2 = softmax(x1_2 @ c3^T);  x3_3 = softmax(x1_2 @ x3_2^T)
            x22 = sm.tile([CH, PIX], BF16, tag="x22")
            product(tx12, tc3, x22, nc.vector)
            x33 = sm.tile([CH, PIX], BF16, tag="x33")
            product(tx12, tx32, x33, nc.vector)

            # ---------- x_f = relu(x3_3 + x2_2 + c3), back to c-layout ------
            nc.vector.tensor_tensor(out=x22[:, :], in0=x22[:, :],
                                    in1=x33[:, :], op=OP.add)
            # bounce through DRAM: write (chi,a),(clo,b) -> (c,(a,b)) order
            xfT = sm.tile([CH, PIX], BF16, tag="xfT")
            for chi in range(2):
                dst = bass.AP(
                    tensor=bT_dram, offset=chi * H2 * PIX,
                    ap=[[H2, H2], [PIX, H2], [1, H2]])  # (a-part, clo, b)
                nc.sync.dma_start(
                    out=dst,
                    in_=x22[chi * H2:(chi + 1) * H2, :])
            nc.sync.dma_start(out=xfT[:, :], in_=bT_dram[:, :])
            xf = sm.tile([CH, PIX], BF16, tag="xf")
            nc.vector.tensor_tensor(out=xf[:, :], in0=xfT[:, :], in1=c3[:, :],
                                    op=OP.add)
            nc.scalar.activation(out=xf[:, :], in_=xf[:, :], func=AF.Relu)

            # ---------- bilinear 2x upsample (half-pixel centers) ----------
            uh = big.tile([CH, H * W2], BF16, tag="uh")      # (CH, 64, 32)
            xv = xf.rearrange("p (h w) -> p h w", h=H2)
            uv = uh.rearrange("p (h w) -> p h w", h=H)
            nc.vector.tensor_copy(out=uv[:, 0, :], in_=xv[:, 0, :])
            nc.vector.tensor_copy(out=uv[:, H - 1, :], in_=xv[:, H2 - 1, :])
            dif = sm.tile([CH, (H2 - 1) * W2], BF16, tag="dif")
            dv = dif.rearrange("p (h w) -> p h w", h=H2 - 1)
            nc.vector.tensor_tensor(out=dv, in0=xv[:, 0:H2 - 1, :],
                                    in1=xv[:, 1:H2, :], op=OP.subtract)
            ev = bass.AP(tensor=uh.tensor, offset=uh.offset + 2 * W2,
                         ap=[list(uh.ap[0]), [2 * W2, H2 - 1], [1, W2]])
            nc.vector.scalar_tensor_tensor(
                out=ev, in0=dv, scalar=0.25, in1=xv[:, 1:H2, :],
                op0=OP.mult, op1=OP.add)
            ov = bass.AP(tensor=uh.tensor, offset=uh.offset + W2,
                         ap=[list(uh.ap[0]), [2 * W2, H2 - 1], [1, W2]])
            nc.vector.scalar_tensor_tensor(
                out=ov, in0=dv, scalar=-0.25, in1=xv[:, 0:H2 - 1, :],
                op0=OP.mult, op1=OP.add)
            # cols (w)
            outsb = big.tile([CH, HW], BF16, tag="outsb")
            ov2 = outsb.rearrange("p (h w) -> p h w", h=H)
            uv2 = uh.rearrange("p (h w) -> p h w", h=H)
            nc.vector.tensor_copy(out=ov2[:, :, 0], in_=uv2[:, :, 0])
            nc.vector.tensor_copy(out=ov2[:, :, W - 1], in_=uv2[:, :, W2 - 1])
            difw = sm.tile([CH, H * (W2 - 1)], BF16, tag="difw")
            dwv = difw.rearrange("p (h w) -> p h w", h=H)
            nc.vector.tensor_tensor(out=dwv, in0=uv2[:, :, 0:W2 - 1],
                                    in1=uv2[:, :, 1:W2], op=OP.subtract)
            evw = bass.AP(tensor=outsb.tensor, offset=outsb.offset + 2,
                          ap=[list(outsb.ap[0]), [W, H], [2, W2 - 1]])
            nc.vector.scalar_tensor_tensor(
                out=evw, in0=dwv, scalar=0.25, in1=uv2[:, :, 1:W2],
                op0=OP.mult, op1=OP.add)
            ovw = bass.AP(tensor=outsb.tensor, offset=outsb.offset + 1,
                          ap=[list(outsb.ap[0]), [W, H], [2, W2 - 1]])
            nc.vector.scalar_tensor_tensor(
                out=ovw, in0=dwv, scalar=-0.25, in1=uv2[:, :, 0:W2 - 1],
                op0=OP.mult, op1=OP.add)

            nc.sync.dma_start(out=out_ext[:, :], in_=outsb[:, :])

    return nc


_NC_CACHE = {}
_LAST_IN_MAPS = None


def kernel(x, w_pos, b_pos, w3, b3):
    x = np.asarray(x, np.float32)
    w_pos = np.asarray(w_pos, np.float32)
    b_pos = np.asarray(b_pos, np.float32)
    w3 = np.asarray(w3, np.float32)
    b3 = np.asarray(b3, np.float32)

    x_ = x[0]                                   # (C, H, W)
    xhw = x_.reshape(C, HW)                     # i = h*W + w
    xwh = x_.transpose(0, 2, 1).reshape(C, HW)  # i = w*H + h
    bf = lambda a: np.ascontiguousarray(a).astype(_BF)  # noqa: E731

    xhw_bf = bf(xhw)
    wposT = bf(w_pos.reshape(C8, C).T)
    bpos = np.ascontiguousarray(b_pos.reshape(C8, 1))
    w3b_all = bf(w3.transpose(1, 2, 3, 0).reshape(C, 9, C))  # (cin, tap, cout)

    in_maps = []
    for m in range(NCORE):
        w3s = w3[m * CH:(m + 1) * CH]           # (CH, C, 3, 3)
        w3t = w3s.transpose(1, 2, 3, 0).reshape(C, 9, CH).reshape(4, 128, 9,
                                                                  CH)
        x5T = xhw[:, m * IB:(m + 1) * IB].T     # (IB, C)
        x5T2 = x5T.reshape(2, 2, 128, C).transpose(0, 2, 1, 3)
        in_maps.append({
            "xhw": xhw_bf,
            "xP": bf(xwh[:, m * IB:(m + 1) * IB]),
            "x5T2": bf(x5T2),
            "xblk": np.ascontiguousarray(xhw[m * CH:(m + 1) * CH, :]),
            "wposT": wposT,
            "bpos": bpos,
            "w3a": bf(w3t),
            "w3b": np.ascontiguousarray(
                w3b_all[m * CH:(m + 1) * CH]),
            "b3": np.ascontiguousarray(b3[m * CH:(m + 1) * CH].reshape(CH,
                                                                       1)),
        })

    global _LAST_IN_MAPS
    _LAST_IN_MAPS = in_maps
    if "nc" not in _NC_CACHE:
        nc_ = _build_real()
        nc_.finalize()
        _NC_CACHE["nc"] = nc_
    nc = _NC_CACHE["nc"]

    res = run_bass_kernel_spmd(nc, in_maps, core_ids=list(range(NCORE)))
    outs = [np.asarray(res.results[m]["out"], np.float32)
            for m in range(NCORE)]
    full = np.concatenate(outs, axis=0).reshape(1, C, H, W)
    return full
